# revision 97
# baseline (speedup 1.0000x reference)
"""Trainium2 Bass kernel for nn_CausalSelfAttention_35931696398729.

Sharding: 8 cores = (batch b in {0,1}) x (kv-head n in {0..3}).
Each core computes its 4 query heads' causal GQA attention for its batch
plus the partial c_proj (rows of Wo for its heads); the host sums the 4
partials per batch.  No device collectives.

Layouts are "transposed" throughout: qT/kT (d on partitions, t free) so
scores come out as ST (keys on partitions, queries free) and PV consumes
exp(ST) directly;  c_proj consumes the attention output OT (d, t) as the
stationary operand with no transposes anywhere except V (16 PE-transposes).

QK RMSNorm is folded in without normalizing q/k tensors elementwise:
 - q-side factor r_q(t)/sqrt(HD) multiplies qT columns (query temperature)
 - k-side factor r_k(s) rides the Exp activation's per-partition scale
 - gamma_q*gamma_k multiplies kT rows (per-partition)
 - softmax runs without max-subtraction (|scores| <= sqrt(HD) after norm)
 - 1/rowsum is applied to OT columns after PV.
"""

import os
import sys
from contextlib import ExitStack

sys.path.insert(0, "/opt/trn_rl_repo")

import numpy as np

import concourse.bacc as bacc
import concourse.mybir as mybir
import concourse.tile as tile
from concourse import bass_utils
from concourse.masks import make_identity

B, T, D = 2, 2048, 2048
NH, NKV, HD = 16, 4, 128
G = NH // NKV  # query heads per core
EPS = 1e-6
THETA = 10000.0
N_CORES = 8
P = 128
TC = 512            # q-chunk for attention / c_proj column chunk
NTC = T // TC       # 4
TC1 = 256           # t-chunk for phase-1 projections
NTC1 = T // TC1     # 8
NKT = D // P        # 16 contraction chunks
NTB = T // P        # 16 t-blocks

MM_MODE = os.environ.get("KERNEL_MM_DT", "bfloat16")
F32 = mybir.dt.float32
MM_DT = {"float32": F32, "float32r": mybir.dt.float32r,
         "bfloat16": mybir.dt.bfloat16}[MM_MODE]
# storage dtype for matmul operands (fp32r operands must be *produced* as
# fp32r for the BIR verifier, so operand tiles use the matmul dtype).
ST_DT = MM_DT
NP_ST = np.dtype("bfloat16") if MM_MODE == "bfloat16" else np.float32


def _mm(ap):
    return ap


def build_program():
    nc = bacc.Bacc("TRN2", target_bir_lowering=False, debug=False,
                   enable_asserts=False, num_devices=N_CORES)

    x_dt = ST_DT
    y_dt = mybir.dt.bfloat16 if MM_MODE == "bfloat16" else F32
    # weights arrive host-prepacked in SBUF layout (partition-major) so each
    # DMA descriptor is a multi-KB contiguous run (no sub-512B penalty).
    xT = nc.dram_tensor("xT", (D, T), x_dt, kind="ExternalInput").ap()
    wq = nc.dram_tensor("wq", (P, G, NKT, HD), x_dt, kind="ExternalInput").ap()
    wk = nc.dram_tensor("wk", (P, NKT, HD), x_dt, kind="ExternalInput").ap()
    wv = nc.dram_tensor("wv", (P, NKT, HD), x_dt, kind="ExternalInput").ap()
    wo = nc.dram_tensor("wo", (P, G, D), ST_DT, kind="ExternalInput").ap()
    tab_dt = mybir.dt.bfloat16 if MM_MODE == "bfloat16" else F32
    tabs = nc.dram_tensor("tabs", (P, 2, T), tab_dt, kind="ExternalInput").ap()
    gamma2 = nc.dram_tensor("gamma2", (P, 1), F32, kind="ExternalInput").ap()
    y = nc.dram_tensor("y", (T, D), y_dt, kind="ExternalOutput").ap()

    with tile.TileContext(nc) as tc, \
         nc.allow_low_precision(reason="fp32r/bf16 matmul operand tiles"):
        with tc.tile_pool(name="persist", bufs=1) as persist, \
             tc.tile_pool(name="stri2", bufs=2) as stri2, \
             tc.tile_pool(name="weights", bufs=1) as wpool, \
             tc.tile_pool(name="xts", bufs=4) as xpool, \
             tc.tile_pool(name="p1tmp", bufs=3) as tmpool:
            tab_sb = persist.tile([P, 2, T], tab_dt)
            cos_sb = tab_sb[:, 0, :]
            sin_sb = tab_sb[:, 1, :]
            g2_sb = persist.tile([P, 1], F32)
            ident_f32 = persist.tile([P, P], F32)
            make_identity(nc, ident_f32)
            ident = persist.tile([P, P], ST_DT)
            nc.vector.tensor_copy(out=ident, in_=ident_f32)
            ones_f32 = persist.tile([P, P], F32)
            nc.vector.memset(ones_f32, 1.0)
            ones_col = persist.tile([P, 1], ST_DT)
            nc.vector.tensor_copy(out=ones_col, in_=ones_f32[:, 0:1])
            ones_st = persist.tile([P, P], ST_DT)
            nc.vector.tensor_copy(out=ones_st, in_=ones_f32)
            eps_k = persist.tile([P, 1], F32)
            nc.vector.memset(eps_k, EPS)
            eps_q = persist.tile([1, 1], F32)
            nc.vector.memset(eps_q, HD * EPS)

            q_sb = [persist.tile([P, T], ST_DT, tag=f"q_sb{h}", name=f"q_sb{h}")
                    for h in range(G)]
            kT_sb = persist.tile([P, T], ST_DT)
            v_sb = persist.tile([P, NTB, P], ST_DT)
            rk_tiles = persist.tile([P, NTB], F32)

            # ---------------- Phase 1: projections + RoPE + norms -----------
            xts_tiles = {}
            sqk_tiles = {}
            with tc.tile_pool(name="p1ps", bufs=3, space="PSUM") as ps_a, \
                 tc.tile_pool(name="p1psv", bufs=1, space="PSUM") as ps_v, \
                 tc.tile_pool(name="p1str", bufs=1, space="PSUM") as ps_s:
                wq_sb = wpool.tile([P, G, NKT, HD], x_dt)
                wk_sb = wpool.tile([P, NKT, HD], x_dt)
                wv_sb = wpool.tile([P, NKT, HD], x_dt)

                def rope_from_psum(dst, ps, sl, make_sq=False):
                    """dst[:, sl] = rope(ps); returns tile holding square."""
                    # ACT drains the psum to a 2-byte tile for the unswapped
                    # ops (2x DVE mode); the half-swapped sin muls must read
                    # the psum directly — the DVE forbids two SBUF inputs at
                    # different base partitions.
                    psb = tmpool.tile([P, TC1], ST_DT, tag="ropecp",
                                      name="ropecp")
                    nc.scalar.copy(out=psb, in_=ps)
                    tmp = tmpool.tile([P, TC1], ST_DT, tag="ropetmp",
                                      name="ropetmp")
                    # tmp = swap(ps) * sinT
                    nc.vector.tensor_mul(out=tmp[0:64, :], in0=ps[64:128, :],
                                         in1=sin_sb[0:64, sl])
                    nc.vector.tensor_mul(out=tmp[64:128, :], in0=ps[0:64, :],
                                         in1=sin_sb[64:128, sl])
                    # dst = psb * cosT + tmp
                    tmp2 = tmpool.tile([P, TC1], ST_DT, tag="ropetmp2",
                                       name="ropetmp2")
                    nc.vector.tensor_mul(out=tmp2, in0=psb, in1=cos_sb[:, sl])
                    nc.vector.tensor_add(out=dst[:, sl], in0=tmp2, in1=tmp)
                    if not make_sq:
                        return None
                    # square for the k-norm (ACT is idle here)
                    sqt = tmpool.tile([P, TC1], F32, tag="ropesq",
                                      name="ropesq")
                    nc.scalar.square(out=sqt, in_=dst[:, sl])
                    return sqt

                def rk_block(ci):
                    # rk column tiles for chunk ci's key blocks:
                    # 1/sqrt(colsum(sq_k)/HD + eps); runs a full chunk after
                    # the square so the PE queue never stalls on it (and so
                    # the tabs DMA may arrive as late as chunk 1)
                    sq_k = sqk_tiles[ci]
                    for i in range(TC1 // P):
                        kb = ci * (TC1 // P) + i
                        ssqc = ps_s.tile([P, 1], F32, tag="ssqc",
                                         name="ssqc")
                        nc.tensor.matmul(ssqc, sq_k[:, i * P:(i + 1) * P],
                                         ones_f32[:, 0:1],
                                         start=True, stop=True)
                        nc.scalar.activation(
                            out=rk_tiles[:, kb:kb + 1], in_=ssqc,
                            func=mybir.ActivationFunctionType.Sqrt,
                            bias=eps_k[:], scale=float(1.0 / HD))
                        nc.vector.reciprocal(out=rk_tiles[:, kb:kb + 1],
                                             in_=rk_tiles[:, kb:kb + 1])

                def load_x(i):
                    xts = xpool.tile([P, NKT, TC1], x_dt, tag="xts",
                                     name="xts")
                    xts_tiles[i] = xts
                    nc.sync.dma_start(
                        out=xts,
                        in_=xT[:, i * TC1:(i + 1) * TC1]
                        .rearrange("(kt p) m -> p kt m", p=P))

                # startup staging, ordered by when PE/rope consume each
                # transfer (HWDGE serializes dma_starts at ~625ns each):
                # x0 (split for an earlier first matmul), k/q/v weights,
                # x1, x2, tables, x3.  tabs must precede the first rope *in
                # program order* but may arrive late: the proj psum is freed
                # by the ACT copy, and the rk ssqc matmuls run two chunks
                # late.
                xts0 = xpool.tile([P, NKT, TC1], x_dt, tag="xts",
                                  name="xts")
                xts_tiles[0] = xts0
                xre0 = xT[:, 0:TC1].rearrange("(kt p) m -> p kt m", p=P)
                nc.sync.dma_start(out=xts0[:, 0:8, :], in_=xre0[:, 0:8, :])
                nc.sync.dma_start(out=wk_sb, in_=wk)
                nc.sync.dma_start(out=xts0[:, 8:16, :], in_=xre0[:, 8:16, :])
                for h in range(G):
                    nc.sync.dma_start(out=wq_sb[:, h], in_=wq[:, h])
                nc.sync.dma_start(out=wv_sb, in_=wv)
                load_x(1)
                nc.sync.dma_start(out=tab_sb, in_=tabs)
                nc.sync.dma_start(out=g2_sb, in_=gamma2)
                load_x(2)
                load_x(3)

                qnorm_tails = {}
                sq_lists = {}
                rq_lists = {}

                def ssq_mm(ci, h):
                    # PE column-sum for chunk ci head h, then sqrt/recip
                    # immediately (ACT / DVE are free here); runs a chunk
                    # after the square so the PE queue never stalls on it
                    ssq = ps_s.tile([1, TC1], F32, tag="ssq",
                                    name="ssq_q", bufs=2)
                    nc.tensor.matmul(ssq, _mm(ones_col),
                                     _mm(sq_lists[ci][h]),
                                     start=True, stop=True)
                    sq_s = stri2.tile([1, TC1], F32, tag="sqs",
                                      name="sq_sq", bufs=4)
                    nc.scalar.activation(
                        out=sq_s, in_=ssq,
                        func=mybir.ActivationFunctionType.Sqrt,
                        bias=eps_q[:], scale=1.0)
                    rq_row = stri2.tile([1, TC1], ST_DT, tag="rqrow",
                                        name="rq_row", bufs=4)
                    nc.vector.reciprocal(out=rq_row, in_=sq_s)
                    rq_lists[ci].append(rq_row)

                for tc_i in range(NTC1):
                    sl = slice(tc_i * TC1, (tc_i + 1) * TC1)
                    qnorm_tail = qnorm_tails.setdefault(tc_i, [])
                    sq_lists[tc_i] = []
                    rq_lists[tc_i] = []
                    if tc_i >= 4:
                        load_x(tc_i)
                    xts = xts_tiles[tc_i]
                    # all 6 projections of the chunk drain (via ACT, to
                    # bf16) into one batch tile; one DMA pair then builds a
                    # half-swapped copy so every rope DVE op below is
                    # all-SBUF bf16 (2x mode, aligned base partitions)
                    psb_all = tmpool.tile([P, 6, TC1], ST_DT, tag="pall",
                                          name="pall", bufs=2)

                    # ---- K ----
                    ps = ps_a.tile([P, TC1], F32, tag="proj", name="ps_k")
                    for kt in range(NKT):
                        nc.tensor.matmul(ps, _mm(wk_sb[:, kt, :]),
                                         _mm(xts[:, kt, :]),
                                         start=(kt == 0), stop=(kt == NKT - 1))
                    nc.scalar.copy(out=psb_all[:, 0, :], in_=ps)

                    # ---- Q heads (prev chunk's norm PE bits interleave) ----
                    for h in range(G):
                        ps = ps_a.tile([P, TC1], F32, tag="proj",
                                       name="ps_q")
                        for kt in range(NKT):
                            nc.tensor.matmul(
                                ps, _mm(wq_sb[:, h, kt, :]),
                                _mm(xts[:, kt, :]),
                                start=(kt == 0), stop=(kt == NKT - 1))
                        nc.scalar.copy(out=psb_all[:, 1 + h, :], in_=ps)
                        if tc_i > 0:
                            ssq_mm(tc_i - 1, h)
                            if h == 0:
                                rk_block(tc_i - 1)

                    # ---- V (chunks 4-7 run at the phase boundary) ----
                    if tc_i < 4:
                        ps = ps_a.tile([P, TC1], F32, tag="proj",
                                       name="ps_vp")
                        for kt in range(NKT):
                            nc.tensor.matmul(ps, _mm(wv_sb[:, kt, :]),
                                             _mm(xts[:, kt, :]),
                                             start=(kt == 0),
                                             stop=(kt == NKT - 1))
                        nc.scalar.copy(out=psb_all[:, 5, :], in_=ps)

                    # ---- batched half-swap ----
                    psw_all = tmpool.tile([P, 6, TC1], ST_DT, tag="pswp",
                                          name="pswp", bufs=2)
                    nc.sync.dma_start(out=psw_all[0:64, :, :],
                                      in_=psb_all[64:128, :, :])
                    nc.sync.dma_start(out=psw_all[64:128, :, :],
                                      in_=psb_all[0:64, :, :])

                    def rope_batch(dst, j, make_sq=False):
                        tmp = tmpool.tile([P, TC1], ST_DT, tag="ropetmp",
                                          name="ropetmp")
                        nc.vector.tensor_mul(out=tmp,
                                             in0=psw_all[:, j, :],
                                             in1=sin_sb[:, sl])
                        tmp2 = tmpool.tile([P, TC1], ST_DT, tag="ropetmp2",
                                           name="ropetmp2")
                        nc.vector.tensor_mul(out=tmp2,
                                             in0=psb_all[:, j, :],
                                             in1=cos_sb[:, sl])
                        nc.vector.tensor_add(out=dst[:, sl], in0=tmp2,
                                             in1=tmp)
                        if not make_sq:
                            return None
                        sqt = tmpool.tile([P, TC1], F32, tag="ropesq",
                                          name="ropesq")
                        nc.scalar.square(out=sqt, in_=dst[:, sl])
                        return sqt

                    sqk_tiles[tc_i] = rope_batch(kT_sb, 0, make_sq=True)
                    # gamma2 applied after the norm-square
                    nc.vector.tensor_scalar_mul(out=kT_sb[:, sl],
                                                in0=kT_sb[:, sl],
                                                scalar1=g2_sb)
                    for h in range(G):
                        rope_batch(q_sb[h], 1 + h)
                        sq_q = tmpool.tile([P, TC1], ST_DT, tag="qsq",
                                           name="sq_q", bufs=5)
                        nc.scalar.square(out=sq_q, in_=q_sb[h][:, sl])
                        sq_lists[tc_i].append(sq_q)

                        def qnorm(h=h, sl=sl, ci=tc_i):
                            rb_ps = ps_s.tile([P, TC1], F32, tag="rqb",
                                              name="rb_ps", bufs=1)
                            nc.tensor.matmul(rb_ps, _mm(ones_st[0:1, :]),
                                             _mm(rq_lists[ci][h]),
                                             start=True, stop=True)
                            nc.vector.tensor_mul(out=q_sb[h][:, sl],
                                                 in0=q_sb[h][:, sl],
                                                 in1=rb_ps)

                        qnorm_tail.append(qnorm)

                    # PE transposes of V read the batch tile directly
                    if tc_i < 4:
                        for i in range(TC1 // P):
                            pst = ps_v.tile([P, P], ST_DT, tag="vtr",
                                            name="pst")
                            nc.tensor.transpose(
                                pst, psb_all[:, 5, i * P:(i + 1) * P],
                                ident)
                            nc.vector.tensor_copy(
                                out=v_sb[:, tc_i * (TC1 // P) + i, :],
                                in_=pst)
                    # previous chunk's temper finalizers
                    for fn_ in qnorm_tails.get(tc_i - 1, []):
                        fn_()
                    if tc_i == NTC1 - 1:
                        for h in range(G):
                            ssq_mm(tc_i, h)
                        rk_block(tc_i)
                        for fn_ in qnorm_tails[tc_i]:
                            fn_()

                # deferred V projections for chunks 4-7 run as a solid PE
                # block at the phase boundary (inside this psum scope, so no
                # cross-phase bank handoff serializes them behind the slow
                # ACT tail); the ACT exp backlog of qc0 drains concurrently.
                for tc_i in range(4, NTC1):
                    ps = ps_a.tile([P, TC1], F32, tag="proj", name="ps_vd")
                    for kt in range(NKT):
                        nc.tensor.matmul(ps, _mm(wv_sb[:, kt, :]),
                                         _mm(xts_tiles[tc_i][:, kt, :]),
                                         start=(kt == 0),
                                         stop=(kt == NKT - 1))
                    vt_sb = tmpool.tile([P, TC1], ST_DT, tag="vt",
                                        name="vt")
                    nc.vector.tensor_copy(out=vt_sb, in_=ps)
                    nc.sync.dma_start_transpose(
                        v_sb[:, 2 * tc_i:2 * tc_i + 2, :], vt_sb)

            # ---------------- Phase 2: attention ---------------------------
            with ExitStack() as p2stack:
                wopool = p2stack.enter_context(
                    tc.tile_pool(name="wo", bufs=1))
                apool = p2stack.enter_context(
                    tc.tile_pool(name="attn", bufs=2))
                ppool = p2stack.enter_context(
                    tc.tile_pool(name="psb", bufs=6))
                otpool = p2stack.enter_context(
                    tc.tile_pool(name="otn", bufs=1))
                wo_sb = wopool.tile([P, G, D], ST_DT)
                for h in range(G):
                    nc.sync.dma_start(out=wo_sb[:, h], in_=wo[:, h])
                otn_sb = [otpool.tile([P, T], ST_DT, tag=f"otn{h}",
                                      name=f"otn{h}")
                          for h in range(G)]

                with ExitStack() as psstack:
                    ps_st = psstack.enter_context(
                        tc.tile_pool(name="p2st", bufs=2, space="PSUM"))
                    ps_ot = psstack.enter_context(
                        tc.tile_pool(name="p2ot", bufs=2, space="PSUM"))
                    ps_rs = psstack.enter_context(
                        tc.tile_pool(name="p2rs", bufs=1, space="PSUM"))
                    ps_rw = psstack.enter_context(
                        tc.tile_pool(name="p2rw", bufs=1, space="PSUM"))
                    ps_ya = psstack.enter_context(
                        tc.tile_pool(name="p3ya", bufs=1, space="PSUM"))
                    ps_yb = psstack.enter_context(
                        tc.tile_pool(name="p3yb", bufs=1, space="PSUM"))
                    ypool = psstack.enter_context(
                        tc.tile_pool(name="ysb", bufs=6))

                    def cproj_gen(qc):
                        """Yield c_proj micro-ops (closures) for qc's four
                        t-blocks; each op is one matmul or the psum->sbuf
                        copies + store DMAs of one (tb, jg) chain."""
                        for tb in range(4 * qc, 4 * qc + 4):
                            for jg in (0, 2):
                                state = {}

                                def op_mm(h, w, tb=tb, jg=jg, state=state):
                                    if h == 0 and w == 0:
                                        state["ya"] = ps_ya.tile(
                                            [P, TC], F32, tag="ya", name="ya")
                                        state["yb"] = ps_yb.tile(
                                            [P, TC], F32, tag="yb", name="yb")
                                    lhs = otn_sb[h][:, tb * P:(tb + 1) * P]
                                    nc.tensor.matmul(
                                        state["ya"] if w == 0
                                        else state["yb"], _mm(lhs),
                                        _mm(wo_sb[:, h,
                                                  (jg + w) * TC:
                                                  (jg + w + 1) * TC]),
                                        start=(h == 0), stop=(h == G - 1))

                                for h in range(G):
                                    for w in (0, 1):
                                        yield (lambda h=h, w=w, f=op_mm:
                                               f(h, w))

                                def op_fin(tb=tb, jg=jg, state=state):
                                    for j, key in ((jg, "ya"), (jg + 1, "yb")):
                                        y_sb = ypool.tile([P, TC], y_dt,
                                                          tag="y_sb",
                                                          name="y_sb")
                                        nc.vector.tensor_copy(out=y_sb,
                                                              in_=state[key])
                                        nc.sync.dma_start(
                                            out=y[tb * P:(tb + 1) * P,
                                                  j * TC:(j + 1) * TC],
                                            in_=y_sb)

                                yield op_fin

                    pending = []

                    def drain(n):
                        # emit up to n pending micro-ops
                        for _ in range(n):
                            if not pending:
                                return
                            pending.pop()()

                    for qc in range(NTC):
                        qsl = slice(qc * TC, (qc + 1) * TC)
                        nkb = 4 * (qc + 1)
                        # drain rate: finish pending ops just as this qc's
                        # attention ends (fractional pacing); qc0 drains
                        # eagerly to ride out the ACT backlog from phase 1
                        rate = len(pending) / float(nkb * G)
                        if qc == 0:
                            rate *= 2.0
                        take_acc = 0.0
                        ots = []
                        # rowsums for all (head, q-subblock) chains land as
                        # psum columns of one [128, 16] tile: p is the
                        # *stationary* operand and a ones-column the moving
                        # one, so each matmul costs ~1 cycle instead of TC.
                        # (full-bank [P, TC] allocation keeps the interleaved
                        # accumulation chains' start-flag zero region private)
                        rs16 = ps_rs.tile([P, TC], F32, tag="rs16",
                                          name="rs16", bufs=1)
                        # zero the chain columns once, then accumulate with
                        # start=False: avoids bank-granular start-flag zeroing
                        # corrupting/serializing the 16 interleaved chains
                        nc.vector.memset(rs16[:, 0:G * 4], 0.0)
                        for h in range(G):
                            ot_ps = ps_ot.tile([P, TC], F32, tag="ot",
                                               name="ot_ps")
                            for kb in range(nkb):
                                r = kb - 4 * qc  # >=0 on diagonal blocks
                                c0 = max(r, 0) * P  # first valid q column
                                st_ps = ps_st.tile([P, TC], F32, tag="st",
                                                   name="st_ps")
                                nc.tensor.matmul(
                                    st_ps[:, c0:],
                                    _mm(kT_sb[:, kb * P:(kb + 1) * P]),
                                    _mm(q_sb[h][:, qc * TC + c0:
                                                (qc + 1) * TC]),
                                    start=True, stop=True)
                                # fill the PE queue *before* the exp-gated
                                # PV matmul so ACT latency is hidden
                                take_acc += rate
                                if take_acc >= 1.0:
                                    n_take = int(take_acc)
                                    take_acc -= n_take
                                    drain(n_take)
                                p_sb = ppool.tile([P, TC], ST_DT, tag="p",
                                                  name="p_sb")
                                nc.scalar.activation(
                                    out=p_sb[:, c0:], in_=st_ps[:, c0:],
                                    func=mybir.ActivationFunctionType.Exp,
                                    scale=rk_tiles[:, kb:kb + 1])
                                if r >= 0:
                                    # causal mask on the diagonal strip only
                                    # (PV/rowsum read cols >= c0): keep iff
                                    # col - p >= 0 within the strip
                                    nc.gpsimd.affine_select(
                                        out=p_sb[:, c0:c0 + P],
                                        in_=p_sb[:, c0:c0 + P],
                                        pattern=[[1, P]],
                                        compare_op=mybir.AluOpType.is_ge,
                                        fill=0.0,
                                        base=0,
                                        channel_multiplier=-1)
                                nc.tensor.matmul(
                                    ot_ps[:, c0:], _mm(v_sb[:, kb, :]),
                                    _mm(p_sb[:, c0:]), start=(kb == 0),
                                    stop=(kb == nkb - 1))
                                for sub in range(max(r, 0), 4):
                                    nc.tensor.matmul(
                                        rs16[:, h * 4 + sub:h * 4 + sub + 1],
                                        _mm(p_sb[:, sub * P:(sub + 1) * P]),
                                        _mm(ones_col),
                                        start=False,
                                        stop=(kb == 4 * qc + sub),
                                        skip_group_check=True)
                            # finalize head h now (per-head, so the ot pool
                            # buffer frees before head h+2 needs it): rowsum
                            # columns -> [4, 128] rows via one PE transpose,
                            # then reciprocal + per-row broadcast + norm.
                            rs4_sb = stri2.tile([P, 4], ST_DT, tag="rs4sb",
                                                name="rs4_sb", bufs=2)
                            nc.vector.tensor_copy(
                                out=rs4_sb, in_=rs16[:, h * 4:h * 4 + 4])
                            # 4 single-column bf16 transposes land every
                            # rowsum row at partition 0 of one [1, TC] psum
                            # row, so the broadcast below is base-0 legal
                            rs_row = ps_rw.tile([1, TC], ST_DT, tag="rsrow",
                                                name="rs_row")
                            for sub in range(4):
                                nc.tensor.matmul(
                                    rs_row[0:1, sub * P:(sub + 1) * P],
                                    rs4_sb[:, sub:sub + 1], ident,
                                    is_transpose=True,
                                    skip_group_check=True)
                            recip_row = stri2.tile([1, TC], ST_DT,
                                                   tag="reciprow",
                                                   name="recip_row", bufs=2)
                            nc.vector.reciprocal(out=recip_row, in_=rs_row)
                            recipB = apool.tile([P, TC], ST_DT, tag="recipB",
                                                name="recipB", bufs=4)
                            nc.gpsimd.partition_broadcast(recipB, recip_row)
                            nc.vector.tensor_mul(out=otn_sb[h][:, qsl],
                                                 in0=ot_ps, in1=recipB)
                            ots.append(ot_ps)

                        # leftover ops from the previous qc, then queue
                        # this qc's c_proj for interleaving into the next
                        # qc's attention (the last qc drains immediately).
                        drain(10 ** 6)
                        pending = list(cproj_gen(qc))
                        pending.reverse()
                    drain(10 ** 6)

    nc.compile()
    return nc


_NC_CACHE = None


def _get_program():
    global _NC_CACHE
    if _NC_CACHE is None:
        _NC_CACHE = build_program()
    return _NC_CACHE


def _make_tables(pos):
    half = HD // 2
    inv_freq = 1.0 / (THETA ** (np.arange(half, dtype=np.float64) / half))
    ang = (pos + np.arange(T, dtype=np.float64))[None, :] * inv_freq[:, None]
    cos = np.cos(ang).astype(np.float32)
    sin = np.sin(ang).astype(np.float32)
    cosT = np.ascontiguousarray(np.concatenate([cos, cos], axis=0))
    sinT = np.ascontiguousarray(np.concatenate([-sin, sin], axis=0))
    return cosT, sinT


def make_in_maps(x, Wq, Wk, Wv, Wo, q_gamma, k_gamma, pos):
    x = np.asarray(x, dtype=np.float32)
    Wq = np.asarray(Wq, dtype=np.float32)
    Wk = np.asarray(Wk, dtype=np.float32)
    Wv = np.asarray(Wv, dtype=np.float32)
    Wo = np.asarray(Wo, dtype=np.float32)
    q_gamma = np.asarray(q_gamma, dtype=np.float32)
    k_gamma = np.asarray(k_gamma, dtype=np.float32)
    pos = int(np.asarray(pos))

    cosT, sinT = _make_tables(pos)
    tabs = np.ascontiguousarray(np.stack([cosT, sinT], axis=1))
    gamma2 = np.ascontiguousarray((q_gamma * k_gamma).reshape(P, 1)
                                  .astype(np.float32))

    def st(a):
        return np.ascontiguousarray(a.astype(NP_ST))

    # prepack weights into the on-device SBUF layouts (partition-major)
    # wq: [D, NH*HD] -> per kv-head n: [p, h, kt, m]
    wq_p = Wq.reshape(NKT, P, NKV, G, HD).transpose(2, 1, 3, 0, 4)
    # wk/wv: [D, NKV*HD] -> per n: [p, kt, m]
    wk_p = Wk.reshape(NKT, P, NKV, HD).transpose(2, 1, 0, 3)
    wv_p = Wv.reshape(NKT, P, NKV, HD).transpose(2, 1, 0, 3)
    # wo: [NH*HD, D] -> per n: [p, h, m]
    wo_p = Wo.reshape(NKV, G, P, D).transpose(0, 2, 1, 3)

    in_maps = []
    for c in range(N_CORES):
        b, n = divmod(c, NKV)
        in_maps.append({
            "xT": st(x[b].T),
            "wq": st(wq_p[n]),
            "wk": st(wk_p[n]),
            "wv": st(wv_p[n]),
            "wo": st(wo_p[n]),
            "tabs": st(tabs),
            "gamma2": gamma2,
        })
    return in_maps


def kernel(x, Wq, Wk, Wv, Wo, q_gamma, k_gamma, pos):
    in_maps = make_in_maps(x, Wq, Wk, Wv, Wo, q_gamma, k_gamma, pos)
    nc = _get_program()
    res = bass_utils.run_bass_kernel_spmd(nc, in_maps,
                                          core_ids=list(range(N_CORES)))
    out = np.zeros((B, T, D), dtype=np.float32)
    for c in range(N_CORES):
        b = c // NKV
        out[b] += np.asarray(res.results[c]["y"], dtype=np.float32)
    return out


if __name__ == "__main__":
    build_program()
    print("program built OK")



# revision 102
# speedup vs baseline: 1.0113x; 1.0113x over previous
"""Trainium2 Bass kernel for nn_CausalSelfAttention_35931696398729.

Sharding: 8 cores = (batch b in {0,1}) x (kv-head n in {0..3}).
Each core computes its 4 query heads' causal GQA attention for its batch
plus the partial c_proj (rows of Wo for its heads); the host sums the 4
partials per batch.  No device collectives.

Layouts are "transposed" throughout: qT/kT (d on partitions, t free) so
scores come out as ST (keys on partitions, queries free) and PV consumes
exp(ST) directly;  c_proj consumes the attention output OT (d, t) as the
stationary operand with no transposes anywhere except V (16 PE-transposes).

QK RMSNorm is folded in without normalizing q/k tensors elementwise:
 - q-side factor r_q(t)/sqrt(HD) multiplies qT columns (query temperature)
 - k-side factor r_k(s) rides the Exp activation's per-partition scale
 - gamma_q*gamma_k multiplies kT rows (per-partition)
 - softmax runs without max-subtraction (|scores| <= sqrt(HD) after norm)
 - 1/rowsum is applied to OT columns after PV.
"""

import os
import sys
from contextlib import ExitStack

sys.path.insert(0, "/opt/trn_rl_repo")

import numpy as np

import concourse.bacc as bacc
import concourse.mybir as mybir
import concourse.tile as tile
from concourse import bass_utils
from concourse.masks import make_identity

B, T, D = 2, 2048, 2048
NH, NKV, HD = 16, 4, 128
G = NH // NKV  # query heads per core
EPS = 1e-6
THETA = 10000.0
N_CORES = 8
P = 128
TC = 512            # q-chunk for attention / c_proj column chunk
NTC = T // TC       # 4
TC1 = 256           # t-chunk for phase-1 projections
NTC1 = T // TC1     # 8
NKT = D // P        # 16 contraction chunks
NTB = T // P        # 16 t-blocks

MM_MODE = os.environ.get("KERNEL_MM_DT", "bfloat16")
F32 = mybir.dt.float32
MM_DT = {"float32": F32, "float32r": mybir.dt.float32r,
         "bfloat16": mybir.dt.bfloat16}[MM_MODE]
# storage dtype for matmul operands (fp32r operands must be *produced* as
# fp32r for the BIR verifier, so operand tiles use the matmul dtype).
ST_DT = MM_DT
NP_ST = np.dtype("bfloat16") if MM_MODE == "bfloat16" else np.float32


def _mm(ap):
    return ap


def build_program():
    nc = bacc.Bacc("TRN2", target_bir_lowering=False, debug=False,
                   enable_asserts=False, num_devices=N_CORES)

    x_dt = ST_DT
    y_dt = mybir.dt.bfloat16 if MM_MODE == "bfloat16" else F32
    # weights arrive host-prepacked in SBUF layout (partition-major) so each
    # DMA descriptor is a multi-KB contiguous run (no sub-512B penalty).
    xT = nc.dram_tensor("xT", (D, T), x_dt, kind="ExternalInput").ap()
    wq = nc.dram_tensor("wq", (P, G, NKT, HD), x_dt, kind="ExternalInput").ap()
    wk = nc.dram_tensor("wk", (P, NKT, HD), x_dt, kind="ExternalInput").ap()
    wv = nc.dram_tensor("wv", (P, NKT, HD), x_dt, kind="ExternalInput").ap()
    wo = nc.dram_tensor("wo", (P, G, D), ST_DT, kind="ExternalInput").ap()
    tab_dt = mybir.dt.bfloat16 if MM_MODE == "bfloat16" else F32
    tabs = nc.dram_tensor("tabs", (P, 2, T), tab_dt, kind="ExternalInput").ap()
    gamma2 = nc.dram_tensor("gamma2", (P, 1), F32, kind="ExternalInput").ap()
    y = nc.dram_tensor("y", (T, D), y_dt, kind="ExternalOutput").ap()

    with tile.TileContext(nc) as tc, \
         nc.allow_low_precision(reason="fp32r/bf16 matmul operand tiles"):
        with tc.tile_pool(name="persist", bufs=1) as persist, \
             tc.tile_pool(name="stri2", bufs=2) as stri2, \
             tc.tile_pool(name="weights", bufs=1) as wpool, \
             tc.tile_pool(name="xts", bufs=4) as xpool, \
             tc.tile_pool(name="p1tmp", bufs=3) as tmpool:
            tab_sb = persist.tile([P, 2, T], tab_dt)
            cos_sb = tab_sb[:, 0, :]
            sin_sb = tab_sb[:, 1, :]
            g2_sb = persist.tile([P, 1], F32)
            ident_f32 = persist.tile([P, P], F32)
            make_identity(nc, ident_f32)
            ident = persist.tile([P, P], ST_DT)
            nc.vector.tensor_copy(out=ident, in_=ident_f32)
            ones_f32 = persist.tile([P, P], F32)
            nc.vector.memset(ones_f32, 1.0)
            ones_col = persist.tile([P, 1], ST_DT)
            nc.vector.tensor_copy(out=ones_col, in_=ones_f32[:, 0:1])
            ones_st = persist.tile([P, P], ST_DT)
            nc.vector.tensor_copy(out=ones_st, in_=ones_f32)
            eps_k = persist.tile([P, 1], F32)
            nc.vector.memset(eps_k, EPS)
            eps_q = persist.tile([1, 1], F32)
            nc.vector.memset(eps_q, HD * EPS)
            # causal triangle mask (keep iff col >= row), built once; the
            # per-diagonal-block masking is then a cheap 2x-mode DVE multiply
            # instead of a Pool affine_select in the exp->PV chain
            tri = persist.tile([P, P], ST_DT)
            nc.vector.memset(tri, 1.0)
            nc.gpsimd.affine_select(out=tri, in_=tri, pattern=[[1, P]],
                                    compare_op=mybir.AluOpType.is_ge,
                                    fill=0.0, base=0, channel_multiplier=-1)

            q_sb = [persist.tile([P, T], ST_DT, tag=f"q_sb{h}", name=f"q_sb{h}")
                    for h in range(G)]
            kT_sb = persist.tile([P, T], ST_DT)
            v_sb = persist.tile([P, NTB, P], ST_DT)
            rk_tiles = persist.tile([P, NTB], F32)

            # ---------------- Phase 1: projections + RoPE + norms -----------
            xts_tiles = {}
            sqk_tiles = {}
            with tc.tile_pool(name="p1ps", bufs=3, space="PSUM") as ps_a, \
                 tc.tile_pool(name="p1psv", bufs=1, space="PSUM") as ps_v, \
                 tc.tile_pool(name="p1str", bufs=1, space="PSUM") as ps_s:
                wq_sb = wpool.tile([P, G, NKT, HD], x_dt)
                wk_sb = wpool.tile([P, NKT, HD], x_dt)
                wv_sb = wpool.tile([P, NKT, HD], x_dt)

                def rope_from_psum(dst, ps, sl, make_sq=False):
                    """dst[:, sl] = rope(ps); returns tile holding square."""
                    # ACT drains the psum to a 2-byte tile for the unswapped
                    # ops (2x DVE mode); the half-swapped sin muls must read
                    # the psum directly — the DVE forbids two SBUF inputs at
                    # different base partitions.
                    psb = tmpool.tile([P, TC1], ST_DT, tag="ropecp",
                                      name="ropecp")
                    nc.scalar.copy(out=psb, in_=ps)
                    tmp = tmpool.tile([P, TC1], ST_DT, tag="ropetmp",
                                      name="ropetmp")
                    # tmp = swap(ps) * sinT
                    nc.vector.tensor_mul(out=tmp[0:64, :], in0=ps[64:128, :],
                                         in1=sin_sb[0:64, sl])
                    nc.vector.tensor_mul(out=tmp[64:128, :], in0=ps[0:64, :],
                                         in1=sin_sb[64:128, sl])
                    # dst = psb * cosT + tmp
                    tmp2 = tmpool.tile([P, TC1], ST_DT, tag="ropetmp2",
                                       name="ropetmp2")
                    nc.vector.tensor_mul(out=tmp2, in0=psb, in1=cos_sb[:, sl])
                    nc.vector.tensor_add(out=dst[:, sl], in0=tmp2, in1=tmp)
                    if not make_sq:
                        return None
                    # square for the k-norm (ACT is idle here)
                    sqt = tmpool.tile([P, TC1], F32, tag="ropesq",
                                      name="ropesq")
                    nc.scalar.square(out=sqt, in_=dst[:, sl])
                    return sqt

                def rk_block(ci):
                    # rk column tiles for chunk ci's key blocks:
                    # 1/sqrt(colsum(sq_k)/HD + eps); runs a full chunk after
                    # the square so the PE queue never stalls on it (and so
                    # the tabs DMA may arrive as late as chunk 1)
                    sq_k = sqk_tiles[ci]
                    for i in range(TC1 // P):
                        kb = ci * (TC1 // P) + i
                        ssqc = ps_s.tile([P, 1], F32, tag="ssqc",
                                         name="ssqc")
                        nc.tensor.matmul(ssqc, sq_k[:, i * P:(i + 1) * P],
                                         ones_f32[:, 0:1],
                                         start=True, stop=True)
                        nc.scalar.activation(
                            out=rk_tiles[:, kb:kb + 1], in_=ssqc,
                            func=mybir.ActivationFunctionType.Sqrt,
                            bias=eps_k[:], scale=float(1.0 / HD))
                        nc.vector.reciprocal(out=rk_tiles[:, kb:kb + 1],
                                             in_=rk_tiles[:, kb:kb + 1])

                def load_x(i):
                    xts = xpool.tile([P, NKT, TC1], x_dt, tag="xts",
                                     name="xts")
                    xts_tiles[i] = xts
                    nc.sync.dma_start(
                        out=xts,
                        in_=xT[:, i * TC1:(i + 1) * TC1]
                        .rearrange("(kt p) m -> p kt m", p=P))

                # startup staging, ordered by when PE/rope consume each
                # transfer (HWDGE serializes dma_starts at ~625ns each):
                # x0 (split for an earlier first matmul), k/q/v weights,
                # x1, x2, tables, x3.  tabs must precede the first rope *in
                # program order* but may arrive late: the proj psum is freed
                # by the ACT copy, and the rk ssqc matmuls run two chunks
                # late.
                xts0 = xpool.tile([P, NKT, TC1], x_dt, tag="xts",
                                  name="xts")
                xts_tiles[0] = xts0
                xre0 = xT[:, 0:TC1].rearrange("(kt p) m -> p kt m", p=P)
                nc.sync.dma_start(out=xts0[:, 0:8, :], in_=xre0[:, 0:8, :])
                nc.sync.dma_start(out=wk_sb, in_=wk)
                nc.sync.dma_start(out=xts0[:, 8:16, :], in_=xre0[:, 8:16, :])
                for h in range(G):
                    nc.sync.dma_start(out=wq_sb[:, h], in_=wq[:, h])
                nc.sync.dma_start(out=wv_sb, in_=wv)
                load_x(1)
                nc.sync.dma_start(out=tab_sb, in_=tabs)
                nc.sync.dma_start(out=g2_sb, in_=gamma2)
                load_x(2)
                load_x(3)

                qnorm_tails = {}
                sq_lists = {}
                rq_lists = {}

                def ssq_mm(ci, h):
                    # PE column-sum for chunk ci head h, then sqrt/recip
                    # immediately (ACT / DVE are free here); runs a chunk
                    # after the square so the PE queue never stalls on it
                    ssq = ps_s.tile([1, TC1], F32, tag="ssq",
                                    name="ssq_q", bufs=2)
                    nc.tensor.matmul(ssq, _mm(ones_col),
                                     _mm(sq_lists[ci][h]),
                                     start=True, stop=True)
                    sq_s = stri2.tile([1, TC1], F32, tag="sqs",
                                      name="sq_sq", bufs=4)
                    nc.scalar.activation(
                        out=sq_s, in_=ssq,
                        func=mybir.ActivationFunctionType.Sqrt,
                        bias=eps_q[:], scale=1.0)
                    rq_row = stri2.tile([1, TC1], ST_DT, tag="rqrow",
                                        name="rq_row", bufs=4)
                    nc.vector.reciprocal(out=rq_row, in_=sq_s)
                    rq_lists[ci].append(rq_row)

                for tc_i in range(NTC1):
                    sl = slice(tc_i * TC1, (tc_i + 1) * TC1)
                    qnorm_tail = qnorm_tails.setdefault(tc_i, [])
                    sq_lists[tc_i] = []
                    rq_lists[tc_i] = []
                    if tc_i >= 4:
                        load_x(tc_i)
                    xts = xts_tiles[tc_i]
                    # all 6 projections of the chunk drain (via ACT, to
                    # bf16) into one batch tile; one DMA pair then builds a
                    # half-swapped copy so every rope DVE op below is
                    # all-SBUF bf16 (2x mode, aligned base partitions)
                    psb_all = tmpool.tile([P, 6, TC1], ST_DT, tag="pall",
                                          name="pall", bufs=2)

                    # ---- K ----
                    ps = ps_a.tile([P, TC1], F32, tag="proj", name="ps_k")
                    for kt in range(NKT):
                        nc.tensor.matmul(ps, _mm(wk_sb[:, kt, :]),
                                         _mm(xts[:, kt, :]),
                                         start=(kt == 0), stop=(kt == NKT - 1))
                    nc.scalar.copy(out=psb_all[:, 0, :], in_=ps)

                    # ---- Q heads (prev chunk's norm PE bits interleave) ----
                    for h in range(G):
                        ps = ps_a.tile([P, TC1], F32, tag="proj",
                                       name="ps_q")
                        for kt in range(NKT):
                            nc.tensor.matmul(
                                ps, _mm(wq_sb[:, h, kt, :]),
                                _mm(xts[:, kt, :]),
                                start=(kt == 0), stop=(kt == NKT - 1))
                        nc.scalar.copy(out=psb_all[:, 1 + h, :], in_=ps)
                        if tc_i > 0:
                            ssq_mm(tc_i - 1, h)
                            if h == 0:
                                rk_block(tc_i - 1)

                    # ---- V (chunks 4-7 run at the phase boundary) ----
                    if tc_i < 4:
                        ps = ps_a.tile([P, TC1], F32, tag="proj",
                                       name="ps_vp")
                        for kt in range(NKT):
                            nc.tensor.matmul(ps, _mm(wv_sb[:, kt, :]),
                                             _mm(xts[:, kt, :]),
                                             start=(kt == 0),
                                             stop=(kt == NKT - 1))
                        nc.scalar.copy(out=psb_all[:, 5, :], in_=ps)

                    # ---- batched half-swap ----
                    psw_all = tmpool.tile([P, 6, TC1], ST_DT, tag="pswp",
                                          name="pswp", bufs=2)
                    nc.sync.dma_start(out=psw_all[0:64, :, :],
                                      in_=psb_all[64:128, :, :])
                    nc.sync.dma_start(out=psw_all[64:128, :, :],
                                      in_=psb_all[0:64, :, :])

                    def rope_batch(dst, j, make_sq=False):
                        tmp = tmpool.tile([P, TC1], ST_DT, tag="ropetmp",
                                          name="ropetmp")
                        nc.vector.tensor_mul(out=tmp,
                                             in0=psw_all[:, j, :],
                                             in1=sin_sb[:, sl])
                        tmp2 = tmpool.tile([P, TC1], ST_DT, tag="ropetmp2",
                                           name="ropetmp2")
                        nc.vector.tensor_mul(out=tmp2,
                                             in0=psb_all[:, j, :],
                                             in1=cos_sb[:, sl])
                        nc.vector.tensor_add(out=dst[:, sl], in0=tmp2,
                                             in1=tmp)
                        if not make_sq:
                            return None
                        sqt = tmpool.tile([P, TC1], F32, tag="ropesq",
                                          name="ropesq")
                        nc.scalar.square(out=sqt, in_=dst[:, sl])
                        return sqt

                    sqk_tiles[tc_i] = rope_batch(kT_sb, 0, make_sq=True)
                    # gamma2 applied after the norm-square
                    nc.vector.tensor_scalar_mul(out=kT_sb[:, sl],
                                                in0=kT_sb[:, sl],
                                                scalar1=g2_sb)
                    for h in range(G):
                        rope_batch(q_sb[h], 1 + h)
                        sq_q = tmpool.tile([P, TC1], ST_DT, tag="qsq",
                                           name="sq_q", bufs=5)
                        nc.scalar.square(out=sq_q, in_=q_sb[h][:, sl])
                        sq_lists[tc_i].append(sq_q)

                        def qnorm(h=h, sl=sl, ci=tc_i):
                            rb_ps = ps_s.tile([P, TC1], F32, tag="rqb",
                                              name="rb_ps", bufs=1)
                            nc.tensor.matmul(rb_ps, _mm(ones_st[0:1, :]),
                                             _mm(rq_lists[ci][h]),
                                             start=True, stop=True)
                            nc.vector.tensor_mul(out=q_sb[h][:, sl],
                                                 in0=q_sb[h][:, sl],
                                                 in1=rb_ps)

                        qnorm_tail.append(qnorm)

                    # PE transposes of V read the batch tile directly
                    if tc_i < 4:
                        for i in range(TC1 // P):
                            pst = ps_v.tile([P, P], ST_DT, tag="vtr",
                                            name="pst")
                            nc.tensor.transpose(
                                pst, psb_all[:, 5, i * P:(i + 1) * P],
                                ident)
                            nc.vector.tensor_copy(
                                out=v_sb[:, tc_i * (TC1 // P) + i, :],
                                in_=pst)
                    # previous chunk's temper finalizers
                    for fn_ in qnorm_tails.get(tc_i - 1, []):
                        fn_()
                    if tc_i == NTC1 - 1:
                        for h in range(G):
                            ssq_mm(tc_i, h)
                        rk_block(tc_i)
                        for fn_ in qnorm_tails[tc_i]:
                            fn_()

                # deferred V projections for chunks 4-7 run as a solid PE
                # block at the phase boundary (inside this psum scope, so no
                # cross-phase bank handoff serializes them behind the slow
                # ACT tail); the ACT exp backlog of qc0 drains concurrently.
                for tc_i in range(4, NTC1):
                    ps = ps_a.tile([P, TC1], F32, tag="proj", name="ps_vd")
                    for kt in range(NKT):
                        nc.tensor.matmul(ps, _mm(wv_sb[:, kt, :]),
                                         _mm(xts_tiles[tc_i][:, kt, :]),
                                         start=(kt == 0),
                                         stop=(kt == NKT - 1))
                    vt_sb = tmpool.tile([P, TC1], ST_DT, tag="vt",
                                        name="vt")
                    nc.vector.tensor_copy(out=vt_sb, in_=ps)
                    nc.sync.dma_start_transpose(
                        v_sb[:, 2 * tc_i:2 * tc_i + 2, :], vt_sb)

            # ---------------- Phase 2: attention ---------------------------
            with ExitStack() as p2stack:
                wopool = p2stack.enter_context(
                    tc.tile_pool(name="wo", bufs=1))
                apool = p2stack.enter_context(
                    tc.tile_pool(name="attn", bufs=2))
                ppool = p2stack.enter_context(
                    tc.tile_pool(name="psb", bufs=6))
                otpool = p2stack.enter_context(
                    tc.tile_pool(name="otn", bufs=1))
                wo_sb = wopool.tile([P, G, D], ST_DT)
                for h in range(G):
                    nc.sync.dma_start(out=wo_sb[:, h], in_=wo[:, h])
                otn_sb = [otpool.tile([P, T], ST_DT, tag=f"otn{h}",
                                      name=f"otn{h}")
                          for h in range(G)]

                with ExitStack() as psstack:
                    ps_st = psstack.enter_context(
                        tc.tile_pool(name="p2st", bufs=2, space="PSUM"))
                    ps_ot = psstack.enter_context(
                        tc.tile_pool(name="p2ot", bufs=2, space="PSUM"))
                    ps_rs = psstack.enter_context(
                        tc.tile_pool(name="p2rs", bufs=1, space="PSUM"))
                    ps_rw = psstack.enter_context(
                        tc.tile_pool(name="p2rw", bufs=1, space="PSUM"))
                    ps_ya = psstack.enter_context(
                        tc.tile_pool(name="p3ya", bufs=1, space="PSUM"))
                    ps_yb = psstack.enter_context(
                        tc.tile_pool(name="p3yb", bufs=1, space="PSUM"))
                    ypool = psstack.enter_context(
                        tc.tile_pool(name="ysb", bufs=6))

                    def cproj_gen(qc):
                        """Yield c_proj micro-ops (closures) for qc's four
                        t-blocks; each op is one matmul or the psum->sbuf
                        copies + store DMAs of one (tb, jg) chain."""
                        for tb in range(4 * qc, 4 * qc + 4):
                            for jg in (0, 2):
                                state = {}

                                def op_mm(h, w, tb=tb, jg=jg, state=state):
                                    if h == 0 and w == 0:
                                        state["ya"] = ps_ya.tile(
                                            [P, TC], F32, tag="ya", name="ya")
                                        state["yb"] = ps_yb.tile(
                                            [P, TC], F32, tag="yb", name="yb")
                                    lhs = otn_sb[h][:, tb * P:(tb + 1) * P]
                                    nc.tensor.matmul(
                                        state["ya"] if w == 0
                                        else state["yb"], _mm(lhs),
                                        _mm(wo_sb[:, h,
                                                  (jg + w) * TC:
                                                  (jg + w + 1) * TC]),
                                        start=(h == 0), stop=(h == G - 1))

                                for h in range(G):
                                    for w in (0, 1):
                                        yield (lambda h=h, w=w, f=op_mm:
                                               f(h, w))

                                def op_fin(tb=tb, jg=jg, state=state):
                                    for j, key in ((jg, "ya"), (jg + 1, "yb")):
                                        y_sb = ypool.tile([P, TC], y_dt,
                                                          tag="y_sb",
                                                          name="y_sb")
                                        nc.vector.tensor_copy(out=y_sb,
                                                              in_=state[key])
                                        nc.sync.dma_start(
                                            out=y[tb * P:(tb + 1) * P,
                                                  j * TC:(j + 1) * TC],
                                            in_=y_sb)

                                yield op_fin

                    pending = []

                    def drain(n):
                        # emit up to n pending micro-ops
                        for _ in range(n):
                            if not pending:
                                return
                            pending.pop()()

                    for qc in range(NTC):
                        qsl = slice(qc * TC, (qc + 1) * TC)
                        nkb = 4 * (qc + 1)
                        # drain rate: finish pending ops just as this qc's
                        # attention ends (fractional pacing); qc0 drains
                        # eagerly to ride out the ACT backlog from phase 1
                        rate = len(pending) / float(nkb * G)
                        if qc == 0:
                            rate *= 2.0
                        take_acc = 0.0
                        ots = []
                        # rowsums for all (head, q-subblock) chains land as
                        # psum columns of one [128, 16] tile: p is the
                        # *stationary* operand and a ones-column the moving
                        # one, so each matmul costs ~1 cycle instead of TC.
                        # (full-bank [P, TC] allocation keeps the interleaved
                        # accumulation chains' start-flag zero region private)
                        rs16 = ps_rs.tile([P, TC], F32, tag="rs16",
                                          name="rs16", bufs=1)
                        # zero the chain columns once, then accumulate with
                        # start=False: avoids bank-granular start-flag zeroing
                        # corrupting/serializing the 16 interleaved chains
                        nc.vector.memset(rs16[:, 0:G * 4], 0.0)
                        for h in range(G):
                            ot_ps = ps_ot.tile([P, TC], F32, tag="ot",
                                               name="ot_ps")
                            for kb in range(nkb):
                                r = kb - 4 * qc  # >=0 on diagonal blocks
                                c0 = max(r, 0) * P  # first valid q column
                                st_ps = ps_st.tile([P, TC], F32, tag="st",
                                                   name="st_ps")
                                nc.tensor.matmul(
                                    st_ps[:, c0:],
                                    _mm(kT_sb[:, kb * P:(kb + 1) * P]),
                                    _mm(q_sb[h][:, qc * TC + c0:
                                                (qc + 1) * TC]),
                                    start=True, stop=True)
                                # fill the PE queue *before* the exp-gated
                                # PV matmul so ACT latency is hidden
                                take_acc += rate
                                if take_acc >= 1.0:
                                    n_take = int(take_acc)
                                    take_acc -= n_take
                                    drain(n_take)
                                p_sb = ppool.tile([P, TC], ST_DT, tag="p",
                                                  name="p_sb")
                                nc.scalar.activation(
                                    out=p_sb[:, c0:], in_=st_ps[:, c0:],
                                    func=mybir.ActivationFunctionType.Exp,
                                    scale=rk_tiles[:, kb:kb + 1])
                                if r >= 0:
                                    # causal mask on the diagonal strip only
                                    # (PV/rowsum read cols >= c0)
                                    nc.vector.tensor_mul(
                                        out=p_sb[:, c0:c0 + P],
                                        in0=p_sb[:, c0:c0 + P],
                                        in1=tri)
                                nc.tensor.matmul(
                                    ot_ps[:, c0:], _mm(v_sb[:, kb, :]),
                                    _mm(p_sb[:, c0:]), start=(kb == 0),
                                    stop=(kb == nkb - 1))
                                for sub in range(max(r, 0), 4):
                                    nc.tensor.matmul(
                                        rs16[:, h * 4 + sub:h * 4 + sub + 1],
                                        _mm(p_sb[:, sub * P:(sub + 1) * P]),
                                        _mm(ones_col),
                                        start=False,
                                        stop=(kb == 4 * qc + sub),
                                        skip_group_check=True)
                            # finalize head h now (per-head, so the ot pool
                            # buffer frees before head h+2 needs it): rowsum
                            # columns -> [4, 128] rows via one PE transpose,
                            # then reciprocal + per-row broadcast + norm.
                            rs4_sb = stri2.tile([P, 4], ST_DT, tag="rs4sb",
                                                name="rs4_sb", bufs=2)
                            nc.vector.tensor_copy(
                                out=rs4_sb, in_=rs16[:, h * 4:h * 4 + 4])
                            # 4 single-column bf16 transposes land every
                            # rowsum row at partition 0 of one [1, TC] psum
                            # row, so the broadcast below is base-0 legal
                            rs_row = ps_rw.tile([1, TC], ST_DT, tag="rsrow",
                                                name="rs_row")
                            for sub in range(4):
                                nc.tensor.matmul(
                                    rs_row[0:1, sub * P:(sub + 1) * P],
                                    rs4_sb[:, sub:sub + 1], ident,
                                    is_transpose=True,
                                    skip_group_check=True)
                            recip_row = stri2.tile([1, TC], ST_DT,
                                                   tag="reciprow",
                                                   name="recip_row", bufs=2)
                            nc.vector.reciprocal(out=recip_row, in_=rs_row)
                            recipB = apool.tile([P, TC], ST_DT, tag="recipB",
                                                name="recipB", bufs=4)
                            nc.gpsimd.partition_broadcast(recipB, recip_row)
                            nc.vector.tensor_mul(out=otn_sb[h][:, qsl],
                                                 in0=ot_ps, in1=recipB)
                            ots.append(ot_ps)

                        # leftover ops from the previous qc, then queue
                        # this qc's c_proj for interleaving into the next
                        # qc's attention (the last qc drains immediately).
                        drain(10 ** 6)
                        pending = list(cproj_gen(qc))
                        pending.reverse()
                    drain(10 ** 6)

    nc.compile()
    return nc


_NC_CACHE = None


def _get_program():
    global _NC_CACHE
    if _NC_CACHE is None:
        _NC_CACHE = build_program()
    return _NC_CACHE


def _make_tables(pos):
    half = HD // 2
    inv_freq = 1.0 / (THETA ** (np.arange(half, dtype=np.float64) / half))
    ang = (pos + np.arange(T, dtype=np.float64))[None, :] * inv_freq[:, None]
    cos = np.cos(ang).astype(np.float32)
    sin = np.sin(ang).astype(np.float32)
    cosT = np.ascontiguousarray(np.concatenate([cos, cos], axis=0))
    sinT = np.ascontiguousarray(np.concatenate([-sin, sin], axis=0))
    return cosT, sinT


def make_in_maps(x, Wq, Wk, Wv, Wo, q_gamma, k_gamma, pos):
    x = np.asarray(x, dtype=np.float32)
    Wq = np.asarray(Wq, dtype=np.float32)
    Wk = np.asarray(Wk, dtype=np.float32)
    Wv = np.asarray(Wv, dtype=np.float32)
    Wo = np.asarray(Wo, dtype=np.float32)
    q_gamma = np.asarray(q_gamma, dtype=np.float32)
    k_gamma = np.asarray(k_gamma, dtype=np.float32)
    pos = int(np.asarray(pos))

    cosT, sinT = _make_tables(pos)
    tabs = np.ascontiguousarray(np.stack([cosT, sinT], axis=1))
    gamma2 = np.ascontiguousarray((q_gamma * k_gamma).reshape(P, 1)
                                  .astype(np.float32))

    def st(a):
        return np.ascontiguousarray(a.astype(NP_ST))

    # prepack weights into the on-device SBUF layouts (partition-major)
    # wq: [D, NH*HD] -> per kv-head n: [p, h, kt, m]
    wq_p = Wq.reshape(NKT, P, NKV, G, HD).transpose(2, 1, 3, 0, 4)
    # wk/wv: [D, NKV*HD] -> per n: [p, kt, m]
    wk_p = Wk.reshape(NKT, P, NKV, HD).transpose(2, 1, 0, 3)
    wv_p = Wv.reshape(NKT, P, NKV, HD).transpose(2, 1, 0, 3)
    # wo: [NH*HD, D] -> per n: [p, h, m]
    wo_p = Wo.reshape(NKV, G, P, D).transpose(0, 2, 1, 3)

    in_maps = []
    for c in range(N_CORES):
        b, n = divmod(c, NKV)
        in_maps.append({
            "xT": st(x[b].T),
            "wq": st(wq_p[n]),
            "wk": st(wk_p[n]),
            "wv": st(wv_p[n]),
            "wo": st(wo_p[n]),
            "tabs": st(tabs),
            "gamma2": gamma2,
        })
    return in_maps


def kernel(x, Wq, Wk, Wv, Wo, q_gamma, k_gamma, pos):
    in_maps = make_in_maps(x, Wq, Wk, Wv, Wo, q_gamma, k_gamma, pos)
    nc = _get_program()
    res = bass_utils.run_bass_kernel_spmd(nc, in_maps,
                                          core_ids=list(range(N_CORES)))
    out = np.zeros((B, T, D), dtype=np.float32)
    for c in range(N_CORES):
        b = c // NKV
        out[b] += np.asarray(res.results[c]["y"], dtype=np.float32)
    return out


if __name__ == "__main__":
    build_program()
    print("program built OK")

